# revision 21
# baseline (speedup 1.0000x reference)
import os
import numpy as np
import ml_dtypes
import bass_rust
from contextlib import ExitStack

from concourse import bacc
import concourse.mybir as mybir
import concourse.tile as tile
from concourse.bass_utils import run_bass_kernel_spmd

BF16 = mybir.dt.bfloat16
FP8 = mybir.dt.float8e4
F32 = mybir.dt.float32
AF = mybir.ActivationFunctionType
OP = mybir.AluOpType

B, C, H, W = 4, 64, 256, 256
HALF = 128
R = 64
FR = 70
EXT = 134
WP = 260
NF = FR * W
NO = R * W
CH = 512

DR_PAIRS = [(i0, j) for j in range(-2, 3) for i0 in (-2, 0)]

_cache = {}


def _build():
    nc = bacc.Bacc(num_devices=8)

    x_d = nc.declare_dram_parameter("x_ext", [C, EXT, W], BF16, isOutput=False)
    m_d = nc.declare_dram_parameter("mask_ext", [C, EXT, W], BF16, isOutput=False)
    xo_d = nc.declare_dram_parameter("x_oth", [C, HALF, W], BF16, isOutput=False)
    lvspa_d = nc.declare_dram_parameter("l_vspa", [128, 128], BF16, isOutput=False)
    lm1_d = nc.declare_dram_parameter("l_m1", [128, 128], BF16, isOutput=False)
    lt_d = nc.declare_dram_parameter("l_t", [128, 128], BF16, isOutput=False)
    lk_d = nc.declare_dram_parameter("l_k", [128, 128], BF16, isOutput=False)
    lout_d = nc.declare_dram_parameter("l_out", [128, 128], BF16, isOutput=False)
    c5p_d = nc.declare_dram_parameter("c5pairs", [10 * 128, 256], FP8,
                                      isOutput=False)
    c5s_d = nc.declare_dram_parameter("c5single", [5 * 128, 128], FP8,
                                      isOutput=False)
    t7_d = nc.declare_dram_parameter("t7", [7 * 128, 64], BF16, isOutput=False)
    wq_d = nc.declare_dram_parameter("wq", [64, 64], F32, isOutput=False)
    lupq_d = nc.declare_dram_parameter("l_upq", [128, 64], F32, isOutput=False)
    wvspe_d = nc.declare_dram_parameter("wv_spe", [64, 64], F32, isOutput=False)
    woutT_d = nc.declare_dram_parameter("woutT", [64, 64], F32, isOutput=False)
    b1_d = nc.declare_dram_parameter("b1_rep", [128, 1], F32, isOutput=False)
    b2_d = nc.declare_dram_parameter("b2_rep", [128, 1], F32, isOutput=False)
    bdw_d = nc.declare_dram_parameter("bdw_rep", [128, 1], F32, isOutput=False)
    rv_d = nc.declare_dram_parameter("rv", [128, 4], F32, isOutput=False)
    ones2_d = nc.declare_dram_parameter("ones2", [2, 128], BF16, isOutput=False)
    b2rv_d = nc.declare_dram_parameter("b2rv", [128, 4], F32, isOutput=False)
    out_d = nc.declare_dram_parameter("out", [128, NO], BF16, isOutput=True)



    with ExitStack() as ctx:
        tc = ctx.enter_context(tile.TileContext(nc))
        pp = ctx.enter_context(tc.tile_pool(name="persist", bufs=1))
        cp = ctx.enter_context(tc.tile_pool(name="chunks", bufs=4))
        ps = ctx.enter_context(tc.tile_pool(name="psum", bufs=2, space="PSUM"))
        psd = ctx.enter_context(tc.tile_pool(name="psumd", bufs=3, space="PSUM"))
        ps1 = ctx.enter_context(tc.tile_pool(name="psum1", bufs=1, space="PSUM"))

        x_sb = pp.tile([128, FR, W], BF16, tag="x_sb")
        m1_sb = pp.tile([128, FR, W], BF16, tag="m1_sb")
        t_sb = pp.tile([128, FR, 272], FP8, tag="t_sb")
        gate_sb2 = pp.tile([2, NO], BF16, tag="gate_sb2")
        ones2 = pp.tile([2, 128], BF16, tag="ones2")

        lvspa = pp.tile([128, 128], BF16, tag="lvspa")
        lm1 = pp.tile([128, 128], BF16, tag="lm1")
        lt = pp.tile([128, 128], BF16, tag="lt")
        lk = pp.tile([128, 128], BF16, tag="lk")
        lout = pp.tile([128, 128], BF16, tag="lout")
        c5p = pp.tile([128, 10, 2, 128], FP8, tag="c5p")
        c5s = pp.tile([128, 5, 128], FP8, tag="c5s")
        t7 = pp.tile([128, 7 * 64], BF16, tag="t7")
        wq_sb = pp.tile([64, 64], F32, tag="wq_sb")
        lupq = pp.tile([128, 64], F32, tag="lupq")
        wvspe = pp.tile([64, 64], F32, tag="wvspe")
        woutT = pp.tile([64, 64], F32, tag="woutT")
        b1 = pp.tile([128, 1], F32, tag="b1")
        b2 = pp.tile([128, 1], F32, tag="b2")
        bdw = pp.tile([128, 1], F32, tag="bdw")
        rv = pp.tile([128, 4], F32, tag="rv")
        b2rv = pp.tile([128, 4], F32, tag="b2rv")
        u_parts = pp.tile([128, 64], F32, tag="u_parts")
        s_parts = pp.tile([128, 64], F32, tag="s_parts")
        leff = pp.tile([128, 128], BF16, tag="leff")
        la = pp.tile([128, 2], BF16, tag="la")
        fr0 = pp.tile([128, W + 6], BF16, tag="fr0")
        fr1 = pp.tile([128, W + 6], BF16, tag="fr1")
        c7s = pp.tile([64, 2 * W], BF16, tag="c7s")

        for sb, d in ((lvspa, lvspa_d), (lm1, lm1_d), (lt, lt_d), (lk, lk_d),
                      (lout, lout_d), (wq_sb, wq_d),
                      (lupq, lupq_d), (wvspe, wvspe_d), (woutT, woutT_d),
                      (b1, b1_d), (b2, b2_d), (bdw, bdw_d), (rv, rv_d),
                      (b2rv, b2rv_d), (ones2, ones2_d)):
            nc.sync.dma_start(sb[:], d[:, :])
        for n in range(10):
            nc.sync.dma_start(
                c5p[:, n],
                c5p_d[n * 128:(n + 1) * 128, :].rearrange("p (q m) -> p q m",
                                                          q=2))
        for n in range(5):
            nc.sync.dma_start(c5s[:, n], c5s_d[n * 128:(n + 1) * 128, :])
        for j in range(7):
            nc.sync.dma_start(t7[:, j * 64:(j + 1) * 64],
                              t7_d[j * 128:(j + 1) * 128, :])

        xv = x_sb
        _eng = [nc.sync, nc.gpsimd, nc.scalar]
        _rg = [(0, 18), (18, 36), (36, 53), (53, FR)]
        for s in range(2):
            for gi, (r0, r1) in enumerate(_rg):
                _eng[gi % 3].dma_start(xv[s * 64:(s + 1) * 64, r0:r1, :],
                                   x_d[:, s * 64 + r0:s * 64 + r1, :])
        x_flat = x_sb.rearrange("p r w -> p (r w)")

        tv = t_sb
        nc.vector.memset(tv[:, :, 0:8], 0.0)
        nc.vector.memset(tv[:, :, W + 8:272], 0.0)

        for k in range(32):
            lo = (3 + 2 * k) * W
            kps = ps.tile([128, CH], F32, tag="mm")
            nc.tensor.matmul(kps[:], lk[:], x_flat[:, lo:lo + CH],
                             start=True, stop=True)
            e_ch = cp.tile([128, CH], BF16, tag="e_ch")
            nc.scalar.activation(e_ch[:], kps[:], AF.Exp,
                                 accum_out=s_parts[:, k:k + 1])
            scr = cp.tile([128, CH], BF16, tag="scr")
            nc.vector.scalar_tensor_tensor(scr[:], x_flat[:, lo:lo + CH], 1.0,
                                           e_ch[:], OP.mult, OP.mult,
                                           accum_out=u_parts[:, k:k + 1])

        for k in range(32):
            xo_ch = cp.tile([128, CH], BF16, tag="xo_ch")
            _e = _eng[k % 3]
            _e.dma_start(xo_ch[0:64], xo_d[:, 2 * k:2 * k + 2, :])
            _e.dma_start(xo_ch[64:128], xo_d[:, 64 + 2 * k:64 + 2 * k + 2, :])
            kps = ps.tile([128, CH], F32, tag="mm")
            nc.tensor.matmul(kps[:], lk[:], xo_ch[:], start=True, stop=True)
            e_ch = cp.tile([128, CH], BF16, tag="e_ch")
            nc.scalar.activation(e_ch[:], kps[:], AF.Exp,
                                 accum_out=s_parts[:, 32 + k:33 + k])
            scr = cp.tile([128, CH], BF16, tag="scr")
            nc.vector.scalar_tensor_tensor(scr[:], xo_ch[:], 1.0,
                                           e_ch[:], OP.mult, OP.mult,
                                           accum_out=u_parts[:, 32 + k:33 + k])
        u_red = pp.tile([128, 1], F32, tag="u_red")
        s_red = pp.tile([128, 1], F32, tag="s_red")
        nc.vector.tensor_reduce(u_red[:], u_parts[:], mybir.AxisListType.X, OP.add)
        nc.vector.tensor_reduce(s_red[:], s_parts[:], mybir.AxisListType.X, OP.add)

        m1_flat = m1_sb.rearrange("p r w -> p (r w)")
        for k in range(35):
            lo = 2 * k * W
            mk = cp.tile([128, CH], BF16, tag="mask_ch")
            _e = _eng[k % 3]
            _e.dma_start(mk[0:64], m_d[:, 2 * k:2 * k + 2, :])
            _e.dma_start(mk[64:128], m_d[:, 64 + 2 * k:64 + 2 * k + 2, :])
            mps = ps.tile([128, CH], F32, tag="mm")
            nc.tensor.matmul(mps[:], lm1[:], mk[:], start=True, stop=True)
            nc.vector.tensor_scalar_add(m1_flat[:, lo:lo + CH], mps[:], b1[:])

        l0ps = ps1.tile([64, 1], F32, tag="small")
        nc.tensor.matmul(l0ps[:], lupq[:], u_red[:], start=True, stop=True)
        l0sb = pp.tile([64, 1], F32, tag="l0sb")
        nc.vector.tensor_copy(l0sb[:], l0ps[:])
        inv64 = pp.tile([128, 1], F32, tag="inv64")
        nc.vector.memset(inv64[:], 1.0 / 64.0)
        s0ps = ps1.tile([1, 1], F32, tag="small")
        nc.tensor.matmul(s0ps[:], inv64[:], s_red[:], start=True, stop=True)
        rs0 = pp.tile([1, 1], F32, tag="rs0")
        nc.vector.reciprocal(rs0[:], s0ps[:])
        ones64 = pp.tile([1, 64], F32, tag="ones64")
        nc.vector.memset(ones64[:], 1.0)
        rbps = ps1.tile([64, 1], F32, tag="small")
        nc.tensor.matmul(rbps[:], ones64[:], rs0[:], start=True, stop=True)
        rb = pp.tile([64, 1], F32, tag="rb")
        nc.vector.tensor_copy(rb[:], rbps[:])
        el = pp.tile([64, 1], F32, tag="el")
        nc.scalar.activation(el[:], l0sb[:], AF.Exp, scale=rb[:])
        ones6464 = pp.tile([64, 64], F32, tag="ones6464")
        nc.vector.memset(ones6464[:], 1.0)
        seps = ps1.tile([64, 1], F32, tag="small")
        nc.tensor.matmul(seps[:], ones6464[:], el[:], start=True, stop=True)
        rsum = pp.tile([64, 1], F32, tag="rsum")
        nc.vector.reciprocal(rsum[:], seps[:])
        an = pp.tile([64, 1], F32, tag="an")
        nc.vector.tensor_tensor(an[:], el[:], rsum[:], OP.mult)
        waps = ps1.tile([64, 1], F32, tag="small")
        nc.tensor.matmul(waps[:], wq_sb[:], an[:], start=True, stop=True)
        wa = pp.tile([64, 1], BF16, tag="wa")
        nc.vector.tensor_copy(wa[:], waps[:])
        nc.vector.memset(la[:], 0.0)
        nc.vector.tensor_copy(la[0:64, 0:1], wa[:])
        nc.vector.tensor_copy(la[64:128, 1:2], wa[:])
        d1 = pp.tile([64, 64], F32, tag="d1")
        nc.vector.tensor_scalar_mul(d1[:], wvspe[:], an[:])
        eps_ = ps1.tile([64, 64], F32, tag="small")
        nc.tensor.matmul(eps_[:], d1[:], woutT[:], start=True, stop=True)
        nc.vector.memset(leff[:], 0.0)
        nc.vector.tensor_copy(leff[0:64, 0:64], eps_[:])
        nc.vector.tensor_copy(leff[64:128, 64:128], eps_[:])

        nc.vector.memset(fr0[:], 0.0)
        nc.vector.memset(fr1[:], 0.0)
        for k in range(35):
            lo = 2 * k * W
            aps_t = ps.tile([2, CH], F32, tag="mm")
            nc.tensor.matmul(aps_t[:], la[:], x_flat[:, lo:lo + CH],
                             start=True, stop=True)
            a_ch = cp.tile([2, CH], BF16, tag="a_ch")
            nc.vector.tensor_copy(a_ch[:], aps_t[:])
            _e2 = _eng[k % 3]
            _e2.dma_start(fr0[2 * k:2 * k + 2, 3:W + 3], a_ch[0:1, :])
            _e2.dma_start(fr1[2 * k:2 * k + 2, 3:W + 3], a_ch[1:2, :])
        for s, frame in ((0, fr0), (1, fr1)):
            c7ps = ps.tile([64, W], F32, tag="mm")
            for j in range(7):
                nc.tensor.matmul(c7ps[:], t7[:, j * 64:(j + 1) * 64],
                                 frame[:, j:j + W],
                                 start=(j == 0), stop=(j == 6))
            nc.scalar.activation(c7s[:, s * W:(s + 1) * W], c7ps[:], AF.Sigmoid)
        nc.sync.dma_start(gate_sb2[0:1, :], c7s[:, 0:W])
        nc.gpsimd.dma_start(gate_sb2[1:2, :], c7s[:, W:2 * W])

        for k in range(34):
            lo = (1 + 2 * k) * W
            tps = ps.tile([128, CH], F32, tag="mm")
            nc.tensor.matmul(tps[:], lt[:], m1_flat[:, lo:lo + CH],
                             start=True, stop=True)
            if k in (0, 33):
                tpv = tps.rearrange("p (r w) -> p r w", w=W)
                for r in range(2):
                    col = r if k == 0 else 2 + r
                    nc.scalar.activation(tv[:, 1 + 2 * k + r, 8:W + 8],
                                         tpv[:, r], AF.Identity,
                                         bias=b2rv[:, col:col + 1],
                                         scale=rv[:, col:col + 1])
            else:
                nc.scalar.activation(tv[:, 1 + 2 * k:3 + 2 * k, 8:W + 8],
                                     tps[:], AF.Identity, bias=b2[:])

        t3 = tv
        for k in range(32):
            fr = 3 + 2 * k
            lo = fr * W
            vps = ps.tile([128, CH], F32, tag="mm")
            nc.tensor.matmul(vps[:], lvspa[:], x_flat[:, lo:lo + CH],
                             start=True, stop=True)
            va = cp.tile([128, CH], BF16, tag="va", bufs=6)
            nc.scalar.activation(va[:], vps[:], AF.Copy)
            dwps = psd.tile([128, 2, W], F32, tag="dw")
            first = True
            for n, (i0, j) in enumerate(DR_PAIRS):
                base = t3[:, fr + i0:fr + i0 + 2, 8 + j:8 + j + W]
                rhs = base.copy()
                _ps = rhs.ap[0][0]
                rhs.ap = bass_rust.VecI64Pair(
                    [(_ps, 128), (272, 2), (272, 2), (1, W)])
                nc.tensor.matmul(dwps[:], c5p[:, n], rhs,
                                 start=first, stop=False,
                                 perf_mode=mybir.MatmulPerfMode.DoubleRow)
                first = False
            for n, j in enumerate(range(-2, 3)):
                nc.tensor.matmul(dwps[:], c5s[:, n],
                                 t3[:, fr + 2:fr + 4, 8 + j:8 + j + W],
                                 start=False, stop=(n == 4))
            am = cp.tile([128, CH], BF16, tag="am", bufs=6)
            nc.scalar.activation(am[:], dwps.rearrange("p a b -> p (a b)"),
                                 AF.Sigmoid, bias=bdw[:])
            gps_t = ps.tile([128, CH], F32, tag="mm")
            nc.tensor.matmul(gps_t[:], ones2[:], gate_sb2[:, k * CH:(k + 1) * CH],
                             start=True, stop=True)
            p2 = cp.tile([128, CH], BF16, tag="p2", bufs=6)
            nc.vector.scalar_tensor_tensor(p2[:], va[:], 1.0, gps_t[:],
                                           OP.mult, OP.mult)
            g1 = cp.tile([128, CH], BF16, tag="g1", bufs=6)
            nc.vector.scalar_tensor_tensor(g1[:], am[:], 1.0,
                                           m1_flat[:, lo:lo + CH],
                                           OP.add, OP.mult)
            p1 = cp.tile([128, CH], BF16, tag="p1", bufs=6)
            nc.vector.tensor_tensor(p1[:], g1[:], va[:], OP.mult)
            ptot = cp.tile([128, CH], BF16, tag="ptot", bufs=6)
            nc.vector.tensor_tensor(ptot[:], p1[:], p2[:], OP.add)
            ops_t = ps.tile([128, CH], F32, tag="out")
            nc.tensor.matmul(ops_t[:], lout[:], ptot[:], start=True, stop=False)
            nc.tensor.matmul(ops_t[:], leff[:], x_flat[:, lo:lo + CH],
                             start=False, stop=True)
            ob = cp.tile([128, CH], BF16, tag="ob")
            nc.scalar.activation(ob[:], ops_t[:], AF.Copy)
            _eng[k % 3].dma_start(out_d[:, k * CH:(k + 1) * CH], ob[:])

    nc.finalize()
    return nc


def _stage(inputs):
    f32 = np.float32
    x = np.asarray(inputs["x"], f32)
    mask = np.asarray(inputs["mask"], f32)
    Wq = np.asarray(inputs["Wq"], f32)
    Wk = np.asarray(inputs["Wk"], f32)
    Wv_spe = np.asarray(inputs["Wv_spe"], f32)
    Wv_spa = np.asarray(inputs["Wv_spa"], f32)
    Wup = np.asarray(inputs["Wup"], f32)
    Wout = np.asarray(inputs["Wout"], f32)
    Wnorm = np.asarray(inputs["Wnorm"], f32)
    mg_w1 = np.asarray(inputs["mg_w1"], f32)
    mg_b1 = np.asarray(inputs["mg_b1"], f32)
    mg_w2 = np.asarray(inputs["mg_w2"], f32)
    mg_b2 = np.asarray(inputs["mg_b2"], f32)
    mg_dw = np.asarray(inputs["mg_dw"], f32)
    mg_bdw = np.asarray(inputs["mg_bdw"], f32)

    bf = ml_dtypes.bfloat16

    def blockdiag(w):
        L = np.zeros((128, 128), f32)
        L[0:64, 0:64] = w.T
        L[64:128, 64:128] = w.T
        return L.astype(bf)

    l_vspa = blockdiag(Wv_spa)
    l_m1 = blockdiag(mg_w1)
    l_t = blockdiag(mg_w2)
    l_out = blockdiag(Wout)
    l_k = np.zeros((128, 128), f32)
    kb = np.tile(Wk[0][:, None], (1, 64))
    l_k[0:64, 0:64] = kb
    l_k[64:128, 64:128] = kb
    l_k = l_k.astype(bf)

    f8 = ml_dtypes.float8_e4m3
    c5p = np.zeros((10 * 128, 256), f32)
    for n, (i0, j) in enumerate(DR_PAIRS):
        for q, i in enumerate((i0, i0 + 1)):
            d = np.tile(mg_dw[:, 0, i + 2, j + 2], 2)
            for k in range(128):
                c5p[n * 128 + k, q * 128 + k] = d[k]
    c5p = c5p.astype(f8)
    c5sg = np.zeros((5 * 128, 128), f32)
    for n, j in enumerate(range(-2, 3)):
        d = np.tile(mg_dw[:, 0, 4, j + 2], 2)
        c5sg[n * 128:(n + 1) * 128] = np.diag(d)
    c5sg = c5sg.astype(f8)

    t7 = np.zeros((7 * 128, 64), f32)
    for j in range(7):
        for m in range(64):
            for i3 in range(7):
                k = m + i3
                if k < FR:
                    t7[j * 128 + k, m] = Wnorm[0, 0, i3, j]

    l_upq = np.zeros((128, 64), f32)
    wupq = (Wup @ Wq).T
    l_upq[0:64] = wupq
    l_upq[64:128] = wupq

    b1_rep = np.tile(mg_b1, 2)[:, None].astype(f32)
    b2_rep = np.tile(mg_b2, 2)[:, None].astype(f32)
    bdw_rep = np.tile(mg_bdw, 2)[:, None].astype(f32)

    o2 = np.zeros((2, 128), f32)
    o2[0, 0:64] = 1.0
    o2[1, 64:128] = 1.0
    o2 = o2.astype(bf)

    shared = {
        "l_vspa": l_vspa, "l_m1": l_m1, "l_t": l_t, "l_k": l_k,
        "l_out": l_out, "c5pairs": c5p, "c5single": c5sg,
        "t7": t7.astype(bf), "wq": Wq.astype(f32), "l_upq": l_upq,
        "wv_spe": Wv_spe.astype(f32), "woutT": Wout.T.copy().astype(f32),
        "b1_rep": b1_rep, "b2_rep": b2_rep, "bdw_rep": bdw_rep,
        "ones2": o2,
    }

    in_maps = []
    for core in range(8):
        b, h = core // 2, core % 2
        r0 = 128 * h - 3
        xe = np.zeros((C, EXT, W), f32)
        me = np.zeros((C, EXT, W), f32)
        lo_img, hi_img = max(r0, 0), min(r0 + EXT, H)
        xe[:, lo_img - r0:hi_img - r0] = x[b, :, lo_img:hi_img]
        me[:, lo_img - r0:hi_img - r0] = mask[b, :, lo_img:hi_img]
        rv = np.ones((128, 4), f32)
        for s in range(2):
            for col, frr in enumerate((1, 2, 67, 68)):
                img_row = 128 * h + 64 * s - 3 + frr
                if not (0 <= img_row < H):
                    rv[s * 64:(s + 1) * 64, col] = 0.0
        m = dict(shared)
        m["x_ext"] = xe.astype(bf)
        oh = 1 - h
        m["x_oth"] = x[b, :, 128 * oh:128 * oh + 128].astype(bf)
        m["mask_ext"] = me.astype(bf)
        m["rv"] = rv
        m["b2rv"] = (b2_rep * rv).astype(f32)
        in_maps.append(m)
    return in_maps


def run(inputs, trace=False):
    if "nc" not in _cache:
        _cache["nc"] = _build()
    in_maps = _stage(inputs)
    res = run_bass_kernel_spmd(_cache["nc"], in_maps, core_ids=list(range(8)),
                               trace=trace)
    out = np.empty((B, C, H, W), np.float32)
    for core in range(8):
        b, h = core // 2, core % 2
        o = np.asarray(res.results[core]["out"], dtype=np.float32)
        o = o.reshape(2, 64, R, W)
        out[b, :, 128 * h:128 * h + 64] = o[0]
        out[b, :, 128 * h + 64:128 * h + 128] = o[1]
    return out, res


def kernel(**inputs) -> np.ndarray:
    out, _ = run(inputs, trace=False)
    return out


# revision 25
# speedup vs baseline: 1.1072x; 1.1072x over previous
import os
import numpy as np
import ml_dtypes
import bass_rust
from contextlib import ExitStack

from concourse import bacc
import concourse.mybir as mybir
import concourse.tile as tile
from concourse.bass_utils import run_bass_kernel_spmd

BF16 = mybir.dt.bfloat16
FP8 = mybir.dt.float8e4
F32 = mybir.dt.float32
AF = mybir.ActivationFunctionType
OP = mybir.AluOpType

B, C, H, W = 4, 64, 256, 256
HALF = 128
R = 64
FR = 70
EXT = 134
WP = 260
NF = FR * W
NO = R * W
CH = 512

DR_PAIRS = [(i0, j) for j in range(-2, 3) for i0 in (-2, 0)]

_cache = {}


def _build():
    nc = bacc.Bacc(num_devices=8)

    x_d = nc.declare_dram_parameter("x_ext", [C, EXT, W], BF16, isOutput=False)
    m_d = nc.declare_dram_parameter("mask_ext", [C, EXT, W], BF16, isOutput=False)
    lvspa_d = nc.declare_dram_parameter("l_vspa", [128, 128], BF16, isOutput=False)
    lm1_d = nc.declare_dram_parameter("l_m1", [128, 128], BF16, isOutput=False)
    lt_d = nc.declare_dram_parameter("l_t", [128, 128], BF16, isOutput=False)
    lk_d = nc.declare_dram_parameter("l_k", [128, 128], BF16, isOutput=False)
    lout_d = nc.declare_dram_parameter("l_out", [128, 128], BF16, isOutput=False)
    c5p_d = nc.declare_dram_parameter("c5pairs", [10 * 128, 256], FP8,
                                      isOutput=False)
    c5s_d = nc.declare_dram_parameter("c5single", [5 * 128, 128], FP8,
                                      isOutput=False)
    t7_d = nc.declare_dram_parameter("t7", [7 * 128, 64], BF16, isOutput=False)
    wq_d = nc.declare_dram_parameter("wq", [64, 64], F32, isOutput=False)
    lupq_d = nc.declare_dram_parameter("l_upq", [128, 64], F32, isOutput=False)
    wvspe_d = nc.declare_dram_parameter("wv_spe", [64, 64], F32, isOutput=False)
    woutT_d = nc.declare_dram_parameter("woutT", [64, 64], F32, isOutput=False)
    b1_d = nc.declare_dram_parameter("b1_rep", [128, 1], F32, isOutput=False)
    b2_d = nc.declare_dram_parameter("b2_rep", [128, 1], F32, isOutput=False)
    bdw_d = nc.declare_dram_parameter("bdw_rep", [128, 1], F32, isOutput=False)
    rv_d = nc.declare_dram_parameter("rv", [128, 4], F32, isOutput=False)
    ones2_d = nc.declare_dram_parameter("ones2", [2, 128], BF16, isOutput=False)
    b2rv_d = nc.declare_dram_parameter("b2rv", [128, 4], F32, isOutput=False)
    out_d = nc.declare_dram_parameter("out", [128, NO], BF16, isOutput=True)

    in_cc = nc.dram_tensor("in_cc", [128, 2], F32)
    out_cc = nc.dram_tensor("out_cc", [128, 2], F32)



    with ExitStack() as ctx:
        tc = ctx.enter_context(tile.TileContext(nc))
        pp = ctx.enter_context(tc.tile_pool(name="persist", bufs=1))
        cp = ctx.enter_context(tc.tile_pool(name="chunks", bufs=4))
        ps = ctx.enter_context(tc.tile_pool(name="psum", bufs=2, space="PSUM"))
        psd = ctx.enter_context(tc.tile_pool(name="psumd", bufs=3, space="PSUM"))
        ps1 = ctx.enter_context(tc.tile_pool(name="psum1", bufs=1, space="PSUM"))

        x_sb = pp.tile([128, FR, W], BF16, tag="x_sb")
        m1_sb = pp.tile([128, FR, W], BF16, tag="m1_sb")
        t_sb = pp.tile([128, FR, 272], FP8, tag="t_sb")
        gate_sb2 = pp.tile([2, NO], BF16, tag="gate_sb2")
        ones2 = pp.tile([2, 128], BF16, tag="ones2")

        lvspa = pp.tile([128, 128], BF16, tag="lvspa")
        lm1 = pp.tile([128, 128], BF16, tag="lm1")
        lt = pp.tile([128, 128], BF16, tag="lt")
        lk = pp.tile([128, 128], BF16, tag="lk")
        lout = pp.tile([128, 128], BF16, tag="lout")
        c5p = pp.tile([128, 10, 2, 128], FP8, tag="c5p")
        c5s = pp.tile([128, 5, 128], FP8, tag="c5s")
        t7 = pp.tile([128, 7 * 64], BF16, tag="t7")
        wq_sb = pp.tile([64, 64], F32, tag="wq_sb")
        lupq = pp.tile([128, 64], F32, tag="lupq")
        wvspe = pp.tile([64, 64], F32, tag="wvspe")
        woutT = pp.tile([64, 64], F32, tag="woutT")
        b1 = pp.tile([128, 1], F32, tag="b1")
        b2 = pp.tile([128, 1], F32, tag="b2")
        bdw = pp.tile([128, 1], F32, tag="bdw")
        rv = pp.tile([128, 4], F32, tag="rv")
        b2rv = pp.tile([128, 4], F32, tag="b2rv")
        s_parts = pp.tile([128, 32], F32, tag="s_parts")
        leff = pp.tile([128, 128], BF16, tag="leff")
        la = pp.tile([128, 2], BF16, tag="la")
        fr0 = pp.tile([128, W + 6], BF16, tag="fr0")
        fr1 = pp.tile([128, W + 6], BF16, tag="fr1")
        c7s = pp.tile([64, 2 * W], BF16, tag="c7s")

        for sb, d in ((lvspa, lvspa_d), (lm1, lm1_d), (lt, lt_d), (lk, lk_d),
                      (lout, lout_d), (wq_sb, wq_d),
                      (lupq, lupq_d), (wvspe, wvspe_d), (woutT, woutT_d),
                      (b1, b1_d), (b2, b2_d), (bdw, bdw_d), (rv, rv_d),
                      (b2rv, b2rv_d), (ones2, ones2_d)):
            nc.sync.dma_start(sb[:], d[:, :])
        for n in range(10):
            nc.sync.dma_start(
                c5p[:, n],
                c5p_d[n * 128:(n + 1) * 128, :].rearrange("p (q m) -> p q m",
                                                          q=2))
        for n in range(5):
            nc.sync.dma_start(c5s[:, n], c5s_d[n * 128:(n + 1) * 128, :])
        for j in range(7):
            nc.sync.dma_start(t7[:, j * 64:(j + 1) * 64],
                              t7_d[j * 128:(j + 1) * 128, :])

        xv = x_sb
        _eng = [nc.sync, nc.gpsimd, nc.scalar]
        _rg = [(0, 18), (18, 36), (36, 53), (53, FR)]
        for s in range(2):
            for gi, (r0, r1) in enumerate(_rg):
                _eng[gi % 3].dma_start(xv[s * 64:(s + 1) * 64, r0:r1, :],
                                   x_d[:, s * 64 + r0:s * 64 + r1, :])
        x_flat = x_sb.rearrange("p r w -> p (r w)")

        tv = t_sb
        nc.vector.memset(tv[:, :, 0:8], 0.0)
        nc.vector.memset(tv[:, :, W + 8:272], 0.0)

        u_acc = pp.tile([128, CH], BF16, tag="u_acc")
        nc.vector.memset(u_acc[:], 0.0)
        for k in range(32):
            lo = (3 + 2 * k) * W
            kps = ps.tile([128, CH], F32, tag="mm")
            nc.tensor.matmul(kps[:], lk[:], x_flat[:, lo:lo + CH],
                             start=True, stop=True)
            e_ch = cp.tile([128, CH], BF16, tag="e_ch")
            nc.scalar.activation(e_ch[:], kps[:], AF.Exp,
                                 accum_out=s_parts[:, k:k + 1])
            scr = cp.tile([128, CH], BF16, tag="scr")
            nc.vector.tensor_tensor(scr[:], x_flat[:, lo:lo + CH], e_ch[:],
                                    OP.mult)
            nc.vector.tensor_tensor(u_acc[:], u_acc[:], scr[:], OP.add)

        u_red = pp.tile([128, 1], F32, tag="u_red")
        s_red2 = pp.tile([128, 1], F32, tag="s_red2")
        nc.vector.tensor_reduce(u_red[:], u_acc[:], mybir.AxisListType.X, OP.add)
        nc.vector.tensor_reduce(s_red2[:], s_parts[:], mybir.AxisListType.X, OP.add)
        cc_sb = pp.tile([128, 2], F32, tag="cc_sb")
        nc.vector.tensor_copy(cc_sb[:, 0:1], u_red[:])
        nc.vector.tensor_copy(cc_sb[:, 1:2], s_red2[:])
        nc.sync.dma_start(in_cc[:, :], cc_sb[:])
        nc.gpsimd.collective_compute(
            "AllReduce", OP.add,
            replica_groups=[[0, 1], [2, 3], [4, 5], [6, 7]],
            ins=[in_cc.ap()], outs=[out_cc.ap()],
        )
        cc2 = pp.tile([128, 2], F32, tag="cc2")
        nc.sync.dma_start(cc2[:], out_cc[:, :])

        m1_flat = m1_sb.rearrange("p r w -> p (r w)")
        for k in range(35):
            lo = 2 * k * W
            mk = cp.tile([128, CH], BF16, tag="mask_ch")
            _e = _eng[k % 3]
            _e.dma_start(mk[0:64], m_d[:, 2 * k:2 * k + 2, :])
            _e.dma_start(mk[64:128], m_d[:, 64 + 2 * k:64 + 2 * k + 2, :])
            mps = ps.tile([128, CH], F32, tag="mm")
            nc.tensor.matmul(mps[:], lm1[:], mk[:], start=True, stop=True)
            nc.vector.tensor_scalar_add(m1_flat[:, lo:lo + CH], mps[:], b1[:])

        l0ps = ps1.tile([64, 1], F32, tag="small")
        nc.tensor.matmul(l0ps[:], lupq[:], cc2[:, 0:1], start=True, stop=True)
        l0sb = pp.tile([64, 1], F32, tag="l0sb")
        nc.vector.tensor_copy(l0sb[:], l0ps[:])
        inv64 = pp.tile([128, 1], F32, tag="inv64")
        nc.vector.memset(inv64[:], 1.0 / 64.0)
        s0ps = ps1.tile([1, 1], F32, tag="small")
        nc.tensor.matmul(s0ps[:], inv64[:], cc2[:, 1:2], start=True, stop=True)
        rs0 = pp.tile([1, 1], F32, tag="rs0")
        nc.vector.reciprocal(rs0[:], s0ps[:])
        ones64 = pp.tile([1, 64], F32, tag="ones64")
        nc.vector.memset(ones64[:], 1.0)
        rbps = ps1.tile([64, 1], F32, tag="small")
        nc.tensor.matmul(rbps[:], ones64[:], rs0[:], start=True, stop=True)
        rb = pp.tile([64, 1], F32, tag="rb")
        nc.vector.tensor_copy(rb[:], rbps[:])
        el = pp.tile([64, 1], F32, tag="el")
        nc.scalar.activation(el[:], l0sb[:], AF.Exp, scale=rb[:])
        ones6464 = pp.tile([64, 64], F32, tag="ones6464")
        nc.vector.memset(ones6464[:], 1.0)
        seps = ps1.tile([64, 1], F32, tag="small")
        nc.tensor.matmul(seps[:], ones6464[:], el[:], start=True, stop=True)
        rsum = pp.tile([64, 1], F32, tag="rsum")
        nc.vector.reciprocal(rsum[:], seps[:])
        an = pp.tile([64, 1], F32, tag="an")
        nc.vector.tensor_tensor(an[:], el[:], rsum[:], OP.mult)
        waps = ps1.tile([64, 1], F32, tag="small")
        nc.tensor.matmul(waps[:], wq_sb[:], an[:], start=True, stop=True)
        wa = pp.tile([64, 1], BF16, tag="wa")
        nc.vector.tensor_copy(wa[:], waps[:])
        nc.vector.memset(la[:], 0.0)
        nc.vector.tensor_copy(la[0:64, 0:1], wa[:])
        nc.vector.tensor_copy(la[64:128, 1:2], wa[:])
        d1 = pp.tile([64, 64], F32, tag="d1")
        nc.vector.tensor_scalar_mul(d1[:], wvspe[:], an[:])
        eps_ = ps1.tile([64, 64], F32, tag="small")
        nc.tensor.matmul(eps_[:], d1[:], woutT[:], start=True, stop=True)
        nc.vector.memset(leff[:], 0.0)
        nc.vector.tensor_copy(leff[0:64, 0:64], eps_[:])
        nc.vector.tensor_copy(leff[64:128, 64:128], eps_[:])

        nc.vector.memset(fr0[:], 0.0)
        nc.vector.memset(fr1[:], 0.0)
        for k in range(35):
            lo = 2 * k * W
            aps_t = ps.tile([2, CH], F32, tag="mm")
            nc.tensor.matmul(aps_t[:], la[:], x_flat[:, lo:lo + CH],
                             start=True, stop=True)
            a_ch = cp.tile([2, CH], BF16, tag="a_ch")
            nc.vector.tensor_copy(a_ch[:], aps_t[:])
            _e2 = _eng[k % 3]
            _e2.dma_start(fr0[2 * k:2 * k + 2, 3:W + 3], a_ch[0:1, :])
            _e2.dma_start(fr1[2 * k:2 * k + 2, 3:W + 3], a_ch[1:2, :])
        for s, frame in ((0, fr0), (1, fr1)):
            c7ps = ps.tile([64, W], F32, tag="mm")
            for j in range(7):
                nc.tensor.matmul(c7ps[:], t7[:, j * 64:(j + 1) * 64],
                                 frame[:, j:j + W],
                                 start=(j == 0), stop=(j == 6))
            nc.scalar.activation(c7s[:, s * W:(s + 1) * W], c7ps[:], AF.Sigmoid)
        nc.sync.dma_start(gate_sb2[0:1, :], c7s[:, 0:W])
        nc.gpsimd.dma_start(gate_sb2[1:2, :], c7s[:, W:2 * W])

        for k in range(34):
            lo = (1 + 2 * k) * W
            tps = ps.tile([128, CH], F32, tag="mm")
            nc.tensor.matmul(tps[:], lt[:], m1_flat[:, lo:lo + CH],
                             start=True, stop=True)
            if k in (0, 33):
                tpv = tps.rearrange("p (r w) -> p r w", w=W)
                for r in range(2):
                    col = r if k == 0 else 2 + r
                    nc.scalar.activation(tv[:, 1 + 2 * k + r, 8:W + 8],
                                         tpv[:, r], AF.Identity,
                                         bias=b2rv[:, col:col + 1],
                                         scale=rv[:, col:col + 1])
            else:
                nc.scalar.activation(tv[:, 1 + 2 * k:3 + 2 * k, 8:W + 8],
                                     tps[:], AF.Identity, bias=b2[:])

        t3 = tv
        for k in range(32):
            fr = 3 + 2 * k
            lo = fr * W
            vps = ps.tile([128, CH], F32, tag="mm")
            nc.tensor.matmul(vps[:], lvspa[:], x_flat[:, lo:lo + CH],
                             start=True, stop=True)
            va = cp.tile([128, CH], BF16, tag="va", bufs=6)
            nc.scalar.activation(va[:], vps[:], AF.Copy)
            dwps = psd.tile([128, 2, W], F32, tag="dw")
            first = True
            for n, (i0, j) in enumerate(DR_PAIRS):
                base = t3[:, fr + i0:fr + i0 + 2, 8 + j:8 + j + W]
                rhs = base.copy()
                _ps = rhs.ap[0][0]
                rhs.ap = bass_rust.VecI64Pair(
                    [(_ps, 128), (272, 2), (272, 2), (1, W)])
                nc.tensor.matmul(dwps[:], c5p[:, n], rhs,
                                 start=first, stop=False,
                                 perf_mode=mybir.MatmulPerfMode.DoubleRow)
                first = False
            for n, j in enumerate(range(-2, 3)):
                nc.tensor.matmul(dwps[:], c5s[:, n],
                                 t3[:, fr + 2:fr + 4, 8 + j:8 + j + W],
                                 start=False, stop=(n == 4))
            am = cp.tile([128, CH], BF16, tag="am", bufs=6)
            nc.scalar.activation(am[:], dwps.rearrange("p a b -> p (a b)"),
                                 AF.Sigmoid, bias=bdw[:])
            gps_t = ps.tile([128, CH], F32, tag="mm")
            nc.tensor.matmul(gps_t[:], ones2[:], gate_sb2[:, k * CH:(k + 1) * CH],
                             start=True, stop=True)
            p2 = cp.tile([128, CH], BF16, tag="p2", bufs=6)
            nc.vector.scalar_tensor_tensor(p2[:], va[:], 1.0, gps_t[:],
                                           OP.mult, OP.mult)
            g1 = cp.tile([128, CH], BF16, tag="g1", bufs=6)
            nc.vector.scalar_tensor_tensor(g1[:], am[:], 1.0,
                                           m1_flat[:, lo:lo + CH],
                                           OP.add, OP.mult)
            p1 = cp.tile([128, CH], BF16, tag="p1", bufs=6)
            nc.vector.tensor_tensor(p1[:], g1[:], va[:], OP.mult)
            ptot = cp.tile([128, CH], BF16, tag="ptot", bufs=6)
            nc.vector.tensor_tensor(ptot[:], p1[:], p2[:], OP.add)
            ops_t = ps.tile([128, CH], F32, tag="out")
            nc.tensor.matmul(ops_t[:], lout[:], ptot[:], start=True, stop=False)
            nc.tensor.matmul(ops_t[:], leff[:], x_flat[:, lo:lo + CH],
                             start=False, stop=True)
            ob = cp.tile([128, CH], BF16, tag="ob")
            nc.scalar.activation(ob[:], ops_t[:], AF.Copy)
            _eng[k % 3].dma_start(out_d[:, k * CH:(k + 1) * CH], ob[:])

    nc.finalize()
    return nc


def _stage(inputs):
    f32 = np.float32
    x = np.asarray(inputs["x"], f32)
    mask = np.asarray(inputs["mask"], f32)
    Wq = np.asarray(inputs["Wq"], f32)
    Wk = np.asarray(inputs["Wk"], f32)
    Wv_spe = np.asarray(inputs["Wv_spe"], f32)
    Wv_spa = np.asarray(inputs["Wv_spa"], f32)
    Wup = np.asarray(inputs["Wup"], f32)
    Wout = np.asarray(inputs["Wout"], f32)
    Wnorm = np.asarray(inputs["Wnorm"], f32)
    mg_w1 = np.asarray(inputs["mg_w1"], f32)
    mg_b1 = np.asarray(inputs["mg_b1"], f32)
    mg_w2 = np.asarray(inputs["mg_w2"], f32)
    mg_b2 = np.asarray(inputs["mg_b2"], f32)
    mg_dw = np.asarray(inputs["mg_dw"], f32)
    mg_bdw = np.asarray(inputs["mg_bdw"], f32)

    bf = ml_dtypes.bfloat16

    def blockdiag(w):
        L = np.zeros((128, 128), f32)
        L[0:64, 0:64] = w.T
        L[64:128, 64:128] = w.T
        return L.astype(bf)

    l_vspa = blockdiag(Wv_spa)
    l_m1 = blockdiag(mg_w1)
    l_t = blockdiag(mg_w2)
    l_out = blockdiag(Wout)
    l_k = np.zeros((128, 128), f32)
    kb = np.tile(Wk[0][:, None], (1, 64))
    l_k[0:64, 0:64] = kb
    l_k[64:128, 64:128] = kb
    l_k = l_k.astype(bf)

    f8 = ml_dtypes.float8_e4m3
    c5p = np.zeros((10 * 128, 256), f32)
    for n, (i0, j) in enumerate(DR_PAIRS):
        for q, i in enumerate((i0, i0 + 1)):
            d = np.tile(mg_dw[:, 0, i + 2, j + 2], 2)
            for k in range(128):
                c5p[n * 128 + k, q * 128 + k] = d[k]
    c5p = c5p.astype(f8)
    c5sg = np.zeros((5 * 128, 128), f32)
    for n, j in enumerate(range(-2, 3)):
        d = np.tile(mg_dw[:, 0, 4, j + 2], 2)
        c5sg[n * 128:(n + 1) * 128] = np.diag(d)
    c5sg = c5sg.astype(f8)

    t7 = np.zeros((7 * 128, 64), f32)
    for j in range(7):
        for m in range(64):
            for i3 in range(7):
                k = m + i3
                if k < FR:
                    t7[j * 128 + k, m] = Wnorm[0, 0, i3, j]

    l_upq = np.zeros((128, 64), f32)
    wupq = (Wup @ Wq).T
    l_upq[0:64] = wupq
    l_upq[64:128] = wupq

    b1_rep = np.tile(mg_b1, 2)[:, None].astype(f32)
    b2_rep = np.tile(mg_b2, 2)[:, None].astype(f32)
    bdw_rep = np.tile(mg_bdw, 2)[:, None].astype(f32)

    o2 = np.zeros((2, 128), f32)
    o2[0, 0:64] = 1.0
    o2[1, 64:128] = 1.0
    o2 = o2.astype(bf)

    shared = {
        "l_vspa": l_vspa, "l_m1": l_m1, "l_t": l_t, "l_k": l_k,
        "l_out": l_out, "c5pairs": c5p, "c5single": c5sg,
        "t7": t7.astype(bf), "wq": Wq.astype(f32), "l_upq": l_upq,
        "wv_spe": Wv_spe.astype(f32), "woutT": Wout.T.copy().astype(f32),
        "b1_rep": b1_rep, "b2_rep": b2_rep, "bdw_rep": bdw_rep,
        "ones2": o2,
    }

    in_maps = []
    for core in range(8):
        b, h = core // 2, core % 2
        r0 = 128 * h - 3
        xe = np.zeros((C, EXT, W), f32)
        me = np.zeros((C, EXT, W), f32)
        lo_img, hi_img = max(r0, 0), min(r0 + EXT, H)
        xe[:, lo_img - r0:hi_img - r0] = x[b, :, lo_img:hi_img]
        me[:, lo_img - r0:hi_img - r0] = mask[b, :, lo_img:hi_img]
        rv = np.ones((128, 4), f32)
        for s in range(2):
            for col, frr in enumerate((1, 2, 67, 68)):
                img_row = 128 * h + 64 * s - 3 + frr
                if not (0 <= img_row < H):
                    rv[s * 64:(s + 1) * 64, col] = 0.0
        m = dict(shared)
        m["x_ext"] = xe.astype(bf)
        m["mask_ext"] = me.astype(bf)
        m["rv"] = rv
        m["b2rv"] = (b2_rep * rv).astype(f32)
        in_maps.append(m)
    return in_maps


def run(inputs, trace=False):
    if "nc" not in _cache:
        _cache["nc"] = _build()
    in_maps = _stage(inputs)
    res = run_bass_kernel_spmd(_cache["nc"], in_maps, core_ids=list(range(8)),
                               trace=trace)
    out = np.empty((B, C, H, W), np.float32)
    for core in range(8):
        b, h = core // 2, core % 2
        o = np.asarray(res.results[core]["out"], dtype=np.float32)
        o = o.reshape(2, 64, R, W)
        out[b, :, 128 * h:128 * h + 64] = o[0]
        out[b, :, 128 * h + 64:128 * h + 128] = o[1]
    return out, res


def kernel(**inputs) -> np.ndarray:
    out, _ = run(inputs, trace=False)
    return out
